# revision 45
# baseline (speedup 1.0000x reference)
"""Trainium2 Bass kernel for NeuroplasticLlama block-sparse adapter (moe_routing).

Contract: kernel(**inputs) takes FULL unsharded inputs (as produced by
setup_inputs) and returns the FULL [4, 4096, 4096] float32 output.

Strategy (data/sequence parallel over 8 cores, 2048 tokens each, 4
macrotiles of T=512 tokens per core):
  - Host ships x twice: bf16 (residual add) and fp8e4m3 (all matmuls),
    each in a macrotile-contiguous [mt][g][p][hl][t] layout so every DMA
    is a plain [128 x 2-4KB] fully-sequential block. Output is stored
    bf16 and upcast on host (bf16 I/O costs ~1.5e-3 rel err vs the
    2e-2 gate).
  - The score matmul is folded into the fp8 DoubleRow z-stream as a 5th
    output chunk (W2*64 in fp8; descaled in the PSUM eviction). A is
    shipped as A*64 fp8 for the same reason.
  - Per macrotile: fused z+scores stream (80 DR matmuls, phase A =
    {s,q0,q1} / phase B = {q2,q3} interleaved per k-pair so mt0 paces
    with its loads), top-3 gating chain on DVE, gates expanded via a
    0/1 matmul, delta = block-diag(Bm) matmul.
  - Residual add hybrid: even h-chunks pre-accumulate x into PSUM with
    an identity matmul and evict via scalar copy; odd h-chunks do the
    add on DVE straight from PSUM. This balances PE/Scalar/DVE.
  - Software-pipelined: pass1(i) [loads, z+s, gating] runs one
    macrotile ahead of pass2(i-1) [expand, delta, add, store].
"""

import sys

if "/opt/trn_rl_repo" not in sys.path:
    sys.path.insert(0, "/opt/trn_rl_repo")

import numpy as np
import ml_dtypes

H = 4096
NB = 128
BLK = 32
R = 4
B = 4
S = 4096
NCORES = 8
TPC = (B * S) // NCORES  # tokens per core = 2048
T = 512                  # tokens per macrotile
NMT = TPC // T           # 4 macrotiles per core
NKT = H // 128           # 32 k-tiles over the hidden dim
NTS = T // 128           # 4 token sub-tiles per macrotile
BIG = 1.0e30
ASCALE = 64.0            # fp8 scaling for A (values ~0.02 are denormal at 1x)
WSCALE = 64.0            # fp8 scaling for W2

TRACE = False            # set by test.py for profiling runs
TRACE_DIR = None
LAST_RESULT = None       # BassKernelResults of the last run

_COMPILED = None


def _build():
    import concourse.bacc as bacc
    import concourse.tile as tile
    from concourse import mybir, masks

    f32 = mybir.dt.float32
    bf16 = mybir.dt.bfloat16
    f8 = mybir.dt.float8e4
    AF = mybir.ActivationFunctionType
    AL = mybir.AluOpType
    AX = mybir.AxisListType
    DR = mybir.MatmulPerfMode.DoubleRow

    nc = bacc.Bacc("TRN2", target_bir_lowering=False, debug=False,
                   num_devices=NCORES)

    xtb_d = nc.dram_tensor("xtb", [NMT * 8 * 128, 4 * T], bf16,
                           kind="ExternalInput")
    xt8_d = nc.dram_tensor("xt8", [NMT * 4 * 128, 8 * T], f8,
                           kind="ExternalInput")
    ah_d = nc.dram_tensor("ah", [128, 4 * NKT * 128], f8, kind="ExternalInput")
    w8_d = nc.dram_tensor("w8", [128, NKT * 128], f8, kind="ExternalInput")
    bpk_d = nc.dram_tensor("bpk", [128, NKT * 128], bf16, kind="ExternalInput")
    e_d = nc.dram_tensor("e", [128, 512], bf16, kind="ExternalInput")
    bias_d = nc.dram_tensor("bias", [128, 5], f32, kind="ExternalInput")
    idb_d = nc.dram_tensor("idb", [128, 128], bf16, kind="ExternalInput")
    ytb_d = nc.dram_tensor("ytb", [NMT * 8 * 128, 4 * T], bf16,
                           kind="ExternalOutput")

    xtb_ap = xtb_d.ap()
    xt8_ap = xt8_d.ap()
    ytb_ap = ytb_d.ap()

    with tile.TileContext(nc) as tc:
        from contextlib import ExitStack
        with ExitStack() as ctx:
            cpool = ctx.enter_context(tc.tile_pool(name="consts", bufs=1))
            xpool = ctx.enter_context(tc.tile_pool(name="xg", bufs=3))
            xbpool = ctx.enter_context(tc.tile_pool(name="xb", bufs=2))
            ypool = ctx.enter_context(tc.tile_pool(name="yg", bufs=4))
            zpool = ctx.enter_context(tc.tile_pool(name="zb", bufs=8))
            gpool = ctx.enter_context(tc.tile_pool(name="gate", bufs=3))
            spool = ctx.enter_context(tc.tile_pool(name="scal", bufs=4))
            pp = ctx.enter_context(tc.tile_pool(name="ps", bufs=2, space="PSUM"))

            # ---- persistent constants ----
            # Split into sub-chains across the three DMA-capable queues so
            # the first matmul's inputs (w8/az0/az1 + xb group 0) land early;
            # consts not needed until phase B / pass2 load via a hook emitted
            # after mt0's x loads (see emit_late_consts).
            az = [cpool.tile([128, NKT * 128], f8, name=f"az{q}",
                             tag=f"az{q}") for q in range(4)]
            w8 = cpool.tile([128, NKT * 128], f8, name="w8", tag="w8")
            bpk = cpool.tile([128, NKT * 128], bf16, name="bpk", tag="bpk")
            esb = cpool.tile([128, 512], bf16, name="esb", tag="esb")
            bias = cpool.tile([128, 5], f32, name="bias", tag="bias")
            identb = cpool.tile([128, 128], bf16, name="identb", tag="identb")
            ident = cpool.tile([128, 128], f32, name="ident", tag="ident")
            masks.make_identity(nc, ident[:])

            hw = NKT * 128 // 2
            ah_ap = ah_d.ap()
            nc.gpsimd.dma_start(az[0][:, :hw], ah_ap[:, :hw])
            nc.gpsimd.dma_start(w8[:, :hw], w8_d.ap()[:, :hw])
            nc.gpsimd.dma_start(az[0][:, hw:], ah_ap[:, hw:2 * hw])
            nc.gpsimd.dma_start(w8[:, hw:], w8_d.ap()[:, hw:2 * hw])
            nc.scalar.dma_start(az[1][:, :hw],
                                ah_ap[:, NKT * 128:NKT * 128 + hw])
            nc.scalar.dma_start(az[1][:, hw:],
                                ah_ap[:, NKT * 128 + hw:2 * NKT * 128])

            def emit_late_consts():
                for hh in range(2):
                    o = 2 * NKT * 128
                    nc.gpsimd.dma_start(az[2][:, hh * hw:(hh + 1) * hw],
                                        ah_ap[:, o + hh * hw:o + (hh + 1) * hw])
                for hh in range(2):
                    o = 3 * NKT * 128
                    nc.sync.dma_start(az[3][:, hh * hw:(hh + 1) * hw],
                                      ah_ap[:, o + hh * hw:o + (hh + 1) * hw])
                nc.scalar.dma_start(bias[:], bias_d.ap()[:])
                nc.sync.dma_start(esb[:], e_d.ap()[:])
                nc.sync.dma_start(identb[:], idb_d.ap()[:])
                for hh in range(4):
                    w = NKT * 128 // 4
                    nc.sync.dma_start(bpk[:, hh * w:(hh + 1) * w],
                                      bpk_d.ap()[:, hh * w:(hh + 1) * w])

            state = {}
            NADD_PE = 16  # of 32 h-chunks: residual add via PE identity mm

            def emit_pass1a(i):
                # ---- loads (bf16 on sync queue, fp8 on gpsimd queue) ----
                xg = xpool.tile([128, 32 * T], bf16, name="xg", tag="xg")

                def load_xtb():
                    for g in range(8):
                        nc.sync.dma_start(
                            xg[:, g * 4 * T:(g + 1) * 4 * T],
                            xtb_ap[(i * 8 + g) * 128:(i * 8 + g + 1) * 128, :])
                if i > 0:
                    load_xtb()
                xb = xbpool.tile([128, 32 * T], f8, name="xb", tag="xb")
                for g2 in range(4):
                    src = xt8_ap[(i * 4 + g2) * 128:(i * 4 + g2 + 1) * 128, :]
                    if i == 0:
                        # split mt0 loads per group, across two queues, for
                        # a faster head
                        for hh in range(2):
                            eng = nc.gpsimd if hh == 0 else nc.scalar
                            base = (g2 * 8 + hh * 4) * T
                            for qq in range(2):
                                eng.dma_start(
                                    xb[:, base + qq * 2 * T:
                                       base + (qq + 1) * 2 * T],
                                    src[:, hh * 4 * T + qq * 2 * T:
                                        hh * 4 * T + (qq + 1) * 2 * T])
                    else:
                        nc.gpsimd.dma_start(
                            xb[:, g2 * 8 * T:(g2 + 1) * 8 * T], src)
                if i == 0:
                    emit_late_consts()
                    load_xtb()
                # ---- fused z+scores fp8 DoubleRow stream ----
                # phase A: {scores, q0, q1}; phase B: {q2, q3}; k2-outer so
                # the PE paces with arriving x chunks on mt0.
                sp = pp.tile([128, T], f32, space="PSUM", name="sp", tag="zp",
                              bufs=3)
                zp = [pp.tile([128, T], f32, space="PSUM", name=f"zp{q}",
                              tag="zp", bufs=3) for q in range(2)]
                for k2 in range(NKT // 2):
                    mov = xb[:, (2 * k2) * T:(2 * k2 + 2) * T] \
                        .rearrange("p (two t) -> p two t", two=2)
                    for stat, ps in ((w8, sp), (az[0], zp[0]), (az[1], zp[1])):
                        nc.tensor.matmul(
                            ps[:],
                            stat[:, k2 * 256:(k2 + 1) * 256]
                            .rearrange("p (two m) -> p two m", two=2),
                            mov,
                            start=(k2 == 0), stop=(k2 == NKT // 2 - 1),
                            perf_mode=DR)
                s_sb = gpool.tile([128, T], f32, name="s_sb", tag="s_sb",
                                  bufs=2)
                nc.scalar.activation(s_sb[:], sp[:], AF.Identity,
                                     bias=bias[:, 4:5], scale=1.0 / WSCALE)
                zbs = []
                for q in range(2):
                    zb = zpool.tile([128, T], bf16, name=f"zb{q}", tag="zb")
                    nc.scalar.activation(zb[:], zp[q][:], AF.Identity,
                                         bias=bias[:, q:q + 1],
                                         scale=1.0 / ASCALE)
                    zbs.append(zb)
                zp2 = [pp.tile([128, T], f32, space="PSUM", name=f"zp{q + 2}",
                               tag="zp", bufs=3) for q in range(2)]
                for k2 in range(NKT // 2):
                    mov = xb[:, (2 * k2) * T:(2 * k2 + 2) * T] \
                        .rearrange("p (two t) -> p two t", two=2)
                    for stat, ps in ((az[2], zp2[0]), (az[3], zp2[1])):
                        nc.tensor.matmul(
                            ps[:],
                            stat[:, k2 * 256:(k2 + 1) * 256]
                            .rearrange("p (two m) -> p two m", two=2),
                            mov,
                            start=(k2 == 0), stop=(k2 == NKT // 2 - 1),
                            perf_mode=DR)

                # ---- score transposes (PE) + stn copies (scalar) ----
                stns = []
                for ts in range(NTS):
                    s_ps = pp.tile([128, 128], f32, space="PSUM", name="s_ps",
                                   tag="tr", bufs=1)
                    nc.tensor.transpose(s_ps[:],
                                        s_sb[:, ts * 128:(ts + 1) * 128],
                                        ident[:])
                    stn = gpool.tile([128, 128], f32, name="stn", tag="stn",
                                     bufs=NTS + 1)
                    nc.scalar.copy(stn[:], s_ps[:])
                    stns.append(stn)
                state[i] = [xg, zbs, zp2, stns]

            def emit_pass1b(i):
                xg, zbs, zp2, stns = state[i]
                # ---- gating chain (DVE) ----
                # scores are shifted > 0 by the host bias, so masking out the
                # current max is just (s < r) * s (top-k/softmax are
                # shift-invariant).
                ggs = []
                for ts in range(NTS):
                    stn = stns[ts]
                    r1 = spool.tile([128, 1], f32, name="r1", tag="r1")
                    nc.vector.reduce_max(r1[:], stn[:], axis=AX.X)
                    s2 = gpool.tile([128, 128], f32, name="s2", tag="s2")
                    nc.vector.scalar_tensor_tensor(s2[:], stn[:], r1[:],
                                                   stn[:], AL.is_lt, AL.mult)
                    r2 = spool.tile([128, 1], f32, name="r2", tag="r2")
                    nc.vector.reduce_max(r2[:], s2[:], axis=AX.X)
                    s3 = gpool.tile([128, 128], f32, name="s3", tag="s3")
                    nc.vector.scalar_tensor_tensor(s3[:], s2[:], r2[:],
                                                   s2[:], AL.is_lt, AL.mult)
                    r3 = spool.tile([128, 1], f32, name="r3", tag="r3")
                    nc.vector.reduce_max(r3[:], s3[:], axis=AX.X)
                    nr1 = spool.tile([128, 1], f32, name="nr1", tag="nr1")
                    nc.vector.tensor_scalar_mul(nr1[:], r1[:], -1.0)
                    ex = gpool.tile([128, 128], f32, name="ex", tag="ex")
                    nc.scalar.activation(ex[:], stn[:], AF.Exp, bias=nr1[:],
                                         scale=1.0)
                    em = gpool.tile([128, 128], f32, name="em", tag="em")
                    zs = spool.tile([128, 1], f32, name="zs", tag="zs")
                    nc.vector.scalar_tensor_tensor(em[:], stn[:], r3[:], ex[:],
                                                   AL.is_ge, AL.mult,
                                                   accum_out=zs[:])
                    rz = spool.tile([128, 1], f32, name="rz", tag="rz")
                    nc.vector.reciprocal(rz[:], zs[:])
                    gg = gpool.tile([128, 128], f32, name="gg", tag="gg",
                                    bufs=NTS + 1)
                    nc.vector.tensor_scalar_mul(gg[:], em[:], rz[:])
                    ggs.append(gg)

                # ---- late z evictions (frees zp bufs for next mt) ----
                for q in range(2):
                    zb = zpool.tile([128, T], bf16, name=f"zb{q + 2}",
                                    tag="zb")
                    nc.scalar.activation(zb[:], zp2[q][:], AF.Identity,
                                         bias=bias[:, q + 2:q + 3],
                                         scale=1.0 / ASCALE)
                    zbs.append(zb)
                state[i] = (xg, zbs, ggs)

            def emit_pass2(i):
                xg, zbs, ggs = state.pop(i)
                npe = 0
                # ---- gate transposes back to [n, t] (bf16) ----
                gt_sb = gpool.tile([128, T], bf16, name="gt_sb", tag="gt_sb",
                                   bufs=2)
                for ts in range(NTS):
                    g_ps = pp.tile([128, 128], f32, space="PSUM", name="g_ps",
                                   tag="tr", bufs=1)
                    nc.tensor.transpose(g_ps[:], ggs[ts][:], ident[:])
                    nc.scalar.copy(gt_sb[:, ts * 128:(ts + 1) * 128], g_ps[:])

                # expands + gate-applies first so the delta stream below never
                # waits on a DVE mul queued behind residual adds
                for q in range(4):
                    gx = pp.tile([128, T], f32, space="PSUM", name="gx",
                                 tag="gx", bufs=1)
                    nc.tensor.matmul(gx[:], esb[:, q * 128:(q + 1) * 128],
                                     gt_sb[:], start=True, stop=True)
                    nc.vector.tensor_mul(zbs[q][:], zbs[q][:], gx[:])
                for q in range(4):
                    for hl in range(8):
                        hc = q * 8 + hl
                        if hl % 4 == 0:
                            yg = ypool.tile([128, 4 * T], bf16, name="yg",
                                            tag="yg")
                        ysl = yg[:, (hl % 4) * T:(hl % 4 + 1) * T]
                        xsl = xg[:, hc * T:(hc + 1) * T]
                        dp = pp.tile([128, T], f32, space="PSUM", name="dp",
                                     tag="dp", bufs=3)
                        # scalar takes the first chunks of each group while
                        # DVE is still busy with the gate multiplies
                        if hl in (0, 1, 4, 6):
                            npe += 1
                            nc.tensor.matmul(dp[:], identb[:], xsl,
                                             start=True, stop=False)
                            nc.tensor.matmul(dp[:],
                                             bpk[:, hc * 128:(hc + 1) * 128],
                                             zbs[q][:], start=False, stop=True)
                            nc.scalar.copy(ysl, dp[:])
                        else:
                            nc.tensor.matmul(dp[:],
                                             bpk[:, hc * 128:(hc + 1) * 128],
                                             zbs[q][:], start=True, stop=True)
                            nc.vector.tensor_add(ysl, xsl, dp[:])
                        if hl % 4 == 3:
                            g = hc // 4
                            row0 = (i * 8 + g) * 128
                            nst = 1
                            if i == NMT - 1:
                                nst = 4 if g == 7 else 2
                            w = 4 * T // nst
                            for hh in range(nst):
                                nc.gpsimd.dma_start(
                                    ytb_ap[row0:row0 + 128,
                                           hh * w:(hh + 1) * w],
                                    yg[:, hh * w:(hh + 1) * w])

            emit_pass1a(0)
            emit_pass1b(0)
            for i in range(1, NMT):
                emit_pass1a(i)
                emit_pass2(i - 1)
                emit_pass1b(i)
            emit_pass2(NMT - 1)

    nc.compile()
    return nc


def _prep_consts(task_emb, task_ids, Wp, bp, centers, A, Bm, adapter_scale):
    scale = float(np.asarray(adapter_scale))
    A_all = np.ascontiguousarray(
        A.transpose(1, 0, 2).reshape(H, NB * R).astype(np.float32))
    W2 = (Wp @ centers.T).astype(np.float32)                     # [H, 128]

    # ah: [p, q, hc, m] = A_all[hc*128+p, q*128+m]*ASCALE, fp8 e4m3
    ah = ((A_all * ASCALE).reshape(NKT, 128, 4, 128).transpose(1, 2, 0, 3)
          .reshape(128, 4 * NKT * 128).astype(ml_dtypes.float8_e4m3))
    ah = np.ascontiguousarray(ah)
    # w8: [p, hc, m] = W2[hc*128+p, m]*WSCALE, fp8 e4m3
    w8 = np.ascontiguousarray(
        (W2 * WSCALE).reshape(NKT, 128, 128).transpose(1, 0, 2)
        .reshape(128, NKT * 128).astype(ml_dtypes.float8_e4m3))

    # block-diag up-projection, K=128 per h-chunk
    bpk = np.zeros((128, NKT * 128), np.float32)
    for hc in range(NKT):
        for mblk in range(4):
            n = hc * 4 + mblk
            for r in range(R):
                row = (hc % 8) * 16 + mblk * 4 + r
                bpk[row, hc * 128 + mblk * 32: hc * 128 + mblk * 32 + 32] = \
                    Bm[n, r, :] * scale
    bpk = bpk.astype(ml_dtypes.bfloat16)

    e_np = (np.arange(128)[:, None] == (np.arange(512)[None, :] // 4)) \
        .astype(ml_dtypes.bfloat16)
    idb = np.eye(128, dtype=ml_dtypes.bfloat16)

    sconst = (bp @ centers.T - 0.5 * (centers ** 2).sum(-1)).astype(np.float32)

    biases = []
    for c in range(NCORES):
        te = task_emb[int(np.asarray(task_ids)[c // 2])].astype(np.float32)
        b5 = np.empty((128, 5), np.float32)
        zoff = te @ A_all                                        # [512]
        for q in range(4):
            b5[:, q] = zoff[q * 128:(q + 1) * 128]
        # +64 shifts scores strictly positive (top-k/softmax invariant);
        # the device gating chain relies on it for cheap max-masking
        b5[:, 4] = te @ W2 + sconst + 64.0
        biases.append(np.ascontiguousarray(b5))
    return ah, w8, bpk, e_np, idb, biases


def kernel(x, task_ids, task_emb, Wp, bp, centers, A, Bm, adapter_scale):
    global _COMPILED, LAST_RESULT
    from concourse import bass_utils

    x = np.asarray(x, dtype=np.float32)
    task_ids = np.asarray(task_ids)
    task_emb = np.asarray(task_emb, dtype=np.float32)
    Wp = np.asarray(Wp, dtype=np.float32)
    bp = np.asarray(bp, dtype=np.float32)
    centers = np.asarray(centers, dtype=np.float32)
    A = np.asarray(A, dtype=np.float32)
    Bm = np.asarray(Bm, dtype=np.float32)

    if _COMPILED is None:
        _COMPILED = _build()
    nc = _COMPILED

    ah, w8, bpk, e_np, idb, biases = _prep_consts(
        task_emb, task_ids, Wp, bp, centers, A, Bm, adapter_scale)

    xf = x.reshape(B * S, H)
    in_maps = []
    for c in range(NCORES):
        xtc = xf[c * TPC:(c + 1) * TPC].T                        # [H, TPC]
        # [mt][g][p][hl][t] with h = g*512 + hl*128 + p
        xtb = np.ascontiguousarray(
            xtc.reshape(8, 4, 128, NMT, T).transpose(3, 0, 2, 1, 4)
            .astype(ml_dtypes.bfloat16).reshape(NMT * 8 * 128, 4 * T))
        # [mt][g2][p][hl8][t] with h = g2*1024 + hl8*128 + p
        xt8 = np.ascontiguousarray(
            xtc.reshape(4, 8, 128, NMT, T).transpose(3, 0, 2, 1, 4)
            .astype(ml_dtypes.float8_e4m3).reshape(NMT * 4 * 128, 8 * T))
        in_maps.append({"xtb": xtb, "xt8": xt8, "ah": ah, "w8": w8,
                        "bpk": bpk, "e": e_np, "bias": biases[c], "idb": idb})

    kwargs = {}
    if TRACE:
        kwargs = dict(trace=True, tmpdir=TRACE_DIR)
    res = bass_utils.run_bass_kernel_spmd(
        nc, in_maps, core_ids=list(range(NCORES)), **kwargs)
    LAST_RESULT = res

    out = np.empty((B * S, H), np.float32)
    for c in range(NCORES):
        ytb = res.results[c]["ytb"]
        yt = (ytb.astype(np.float32).reshape(NMT, 8, 128, 4, T)
              .transpose(1, 3, 2, 0, 4).reshape(H, TPC))
        out[c * TPC:(c + 1) * TPC] = yt.T
    return out.reshape(B, S, H)


# revision 46
# speedup vs baseline: 1.0147x; 1.0147x over previous
"""Trainium2 Bass kernel for NeuroplasticLlama block-sparse adapter (moe_routing).

Contract: kernel(**inputs) takes FULL unsharded inputs (as produced by
setup_inputs) and returns the FULL [4, 4096, 4096] float32 output.

Strategy (data/sequence parallel over 8 cores, 2048 tokens each, 4
macrotiles of T=512 tokens per core):
  - Host ships x twice: bf16 (residual add) and fp8e4m3 (all matmuls),
    each in a macrotile-contiguous [mt][g][p][hl][t] layout so every DMA
    is a plain [128 x 2-4KB] fully-sequential block. Output is stored
    bf16 and upcast on host (bf16 I/O costs ~1.5e-3 rel err vs the
    2e-2 gate).
  - The score matmul is folded into the fp8 DoubleRow z-stream as a 5th
    output chunk (W2*64 in fp8; descaled in the PSUM eviction). A is
    shipped as A*64 fp8 for the same reason.
  - Per macrotile: fused z+scores stream (80 DR matmuls, phase A =
    {s,q0,q1} / phase B = {q2,q3} interleaved per k-pair so mt0 paces
    with its loads), top-3 gating chain on DVE, gates expanded via a
    0/1 matmul, delta = block-diag(Bm) matmul.
  - Residual add hybrid: even h-chunks pre-accumulate x into PSUM with
    an identity matmul and evict via scalar copy; odd h-chunks do the
    add on DVE straight from PSUM. This balances PE/Scalar/DVE.
  - Software-pipelined: pass1(i) [loads, z+s, gating] runs one
    macrotile ahead of pass2(i-1) [expand, delta, add, store].
"""

import sys

if "/opt/trn_rl_repo" not in sys.path:
    sys.path.insert(0, "/opt/trn_rl_repo")

import numpy as np
import ml_dtypes

H = 4096
NB = 128
BLK = 32
R = 4
B = 4
S = 4096
NCORES = 8
TPC = (B * S) // NCORES  # tokens per core = 2048
T = 512                  # tokens per macrotile
NMT = TPC // T           # 4 macrotiles per core
NKT = H // 128           # 32 k-tiles over the hidden dim
NTS = T // 128           # 4 token sub-tiles per macrotile
BIG = 1.0e30
ASCALE = 64.0            # fp8 scaling for A (values ~0.02 are denormal at 1x)
WSCALE = 64.0            # fp8 scaling for W2

TRACE = False            # set by test.py for profiling runs
TRACE_DIR = None
LAST_RESULT = None       # BassKernelResults of the last run

_COMPILED = None


def _build():
    import concourse.bacc as bacc
    import concourse.tile as tile
    from concourse import mybir, masks

    f32 = mybir.dt.float32
    bf16 = mybir.dt.bfloat16
    f8 = mybir.dt.float8e4
    AF = mybir.ActivationFunctionType
    AL = mybir.AluOpType
    AX = mybir.AxisListType
    DR = mybir.MatmulPerfMode.DoubleRow

    nc = bacc.Bacc("TRN2", target_bir_lowering=False, debug=False,
                   num_devices=NCORES)

    xtb_d = nc.dram_tensor("xtb", [NMT * 8 * 128, 4 * T], bf16,
                           kind="ExternalInput")
    xt8_d = nc.dram_tensor("xt8", [NMT * 4 * 128, 8 * T], f8,
                           kind="ExternalInput")
    ah_d = nc.dram_tensor("ah", [128, 4 * NKT * 128], f8, kind="ExternalInput")
    w8_d = nc.dram_tensor("w8", [128, NKT * 128], f8, kind="ExternalInput")
    bpk_d = nc.dram_tensor("bpk", [128, NKT * 128], bf16, kind="ExternalInput")
    e_d = nc.dram_tensor("e", [128, 512], bf16, kind="ExternalInput")
    bias_d = nc.dram_tensor("bias", [128, 5], f32, kind="ExternalInput")
    idb_d = nc.dram_tensor("idb", [128, 128], bf16, kind="ExternalInput")
    ytb_d = nc.dram_tensor("ytb", [NMT * 8 * 128, 4 * T], bf16,
                           kind="ExternalOutput")

    xtb_ap = xtb_d.ap()
    xt8_ap = xt8_d.ap()
    ytb_ap = ytb_d.ap()

    with tile.TileContext(nc) as tc:
        from contextlib import ExitStack
        with ExitStack() as ctx:
            cpool = ctx.enter_context(tc.tile_pool(name="consts", bufs=1))
            xpool = ctx.enter_context(tc.tile_pool(name="xg", bufs=3))
            xbpool = ctx.enter_context(tc.tile_pool(name="xb", bufs=2))
            ypool = ctx.enter_context(tc.tile_pool(name="yg", bufs=4))
            zpool = ctx.enter_context(tc.tile_pool(name="zb", bufs=8))
            gpool = ctx.enter_context(tc.tile_pool(name="gate", bufs=3))
            spool = ctx.enter_context(tc.tile_pool(name="scal", bufs=4))
            pp = ctx.enter_context(tc.tile_pool(name="ps", bufs=2, space="PSUM"))

            # ---- persistent constants ----
            # Split into sub-chains across the three DMA-capable queues so
            # the first matmul's inputs (w8/az0/az1 + xb group 0) land early;
            # consts not needed until phase B / pass2 load via a hook emitted
            # after mt0's x loads (see emit_late_consts).
            az = [cpool.tile([128, NKT * 128], f8, name=f"az{q}",
                             tag=f"az{q}") for q in range(4)]
            w8 = cpool.tile([128, NKT * 128], f8, name="w8", tag="w8")
            bpk = cpool.tile([128, NKT * 128], bf16, name="bpk", tag="bpk")
            esb = cpool.tile([128, 512], bf16, name="esb", tag="esb")
            bias = cpool.tile([128, 5], f32, name="bias", tag="bias")
            identb = cpool.tile([128, 128], bf16, name="identb", tag="identb")
            ident = cpool.tile([128, 128], f32, name="ident", tag="ident")
            masks.make_identity(nc, ident[:])

            hw = NKT * 128 // 2
            ah_ap = ah_d.ap()
            nc.gpsimd.dma_start(az[0][:, :hw], ah_ap[:, :hw])
            nc.gpsimd.dma_start(w8[:, :hw], w8_d.ap()[:, :hw])
            nc.gpsimd.dma_start(az[0][:, hw:], ah_ap[:, hw:2 * hw])
            nc.gpsimd.dma_start(w8[:, hw:], w8_d.ap()[:, hw:2 * hw])
            nc.scalar.dma_start(az[1][:, :hw],
                                ah_ap[:, NKT * 128:NKT * 128 + hw])
            nc.scalar.dma_start(az[1][:, hw:],
                                ah_ap[:, NKT * 128 + hw:2 * NKT * 128])

            def emit_late_consts():
                for hh in range(2):
                    o = 2 * NKT * 128
                    nc.gpsimd.dma_start(az[2][:, hh * hw:(hh + 1) * hw],
                                        ah_ap[:, o + hh * hw:o + (hh + 1) * hw])
                for hh in range(2):
                    o = 3 * NKT * 128
                    nc.sync.dma_start(az[3][:, hh * hw:(hh + 1) * hw],
                                      ah_ap[:, o + hh * hw:o + (hh + 1) * hw])
                nc.scalar.dma_start(bias[:], bias_d.ap()[:])
                nc.sync.dma_start(esb[:], e_d.ap()[:])
                nc.sync.dma_start(identb[:], idb_d.ap()[:])
                for hh in range(4):
                    w = NKT * 128 // 4
                    nc.sync.dma_start(bpk[:, hh * w:(hh + 1) * w],
                                      bpk_d.ap()[:, hh * w:(hh + 1) * w])

            state = {}
            NADD_PE = 16  # of 32 h-chunks: residual add via PE identity mm

            def emit_pass1a(i):
                # ---- loads (bf16 on sync queue, fp8 on gpsimd queue) ----
                xg = xpool.tile([128, 32 * T], bf16, name="xg", tag="xg")

                def load_xtb():
                    for g in range(8):
                        nc.sync.dma_start(
                            xg[:, g * 4 * T:(g + 1) * 4 * T],
                            xtb_ap[(i * 8 + g) * 128:(i * 8 + g + 1) * 128, :])
                if i > 0:
                    load_xtb()
                xb = xbpool.tile([128, 32 * T], f8, name="xb", tag="xb")
                for g2 in range(4):
                    src = xt8_ap[(i * 4 + g2) * 128:(i * 4 + g2 + 1) * 128, :]
                    if i == 0:
                        # split mt0 loads per group, across two queues, for
                        # a faster head
                        for hh in range(2):
                            eng = nc.gpsimd if hh == 0 else nc.scalar
                            eng.dma_start(
                                xb[:, (g2 * 8 + hh * 4) * T:
                                   (g2 * 8 + hh * 4 + 4) * T],
                                src[:, hh * 4 * T:(hh + 1) * 4 * T])
                    else:
                        nc.gpsimd.dma_start(
                            xb[:, g2 * 8 * T:(g2 + 1) * 8 * T], src)
                if i == 0:
                    emit_late_consts()
                    load_xtb()
                # ---- fused z+scores fp8 DoubleRow stream ----
                # phase A: {scores, q0, q1}; phase B: {q2, q3}; k2-outer so
                # the PE paces with arriving x chunks on mt0.
                sp = pp.tile([128, T], f32, space="PSUM", name="sp", tag="zp",
                              bufs=3)
                zp = [pp.tile([128, T], f32, space="PSUM", name=f"zp{q}",
                              tag="zp", bufs=3) for q in range(2)]
                for k2 in range(NKT // 2):
                    mov = xb[:, (2 * k2) * T:(2 * k2 + 2) * T] \
                        .rearrange("p (two t) -> p two t", two=2)
                    for stat, ps in ((w8, sp), (az[0], zp[0]), (az[1], zp[1])):
                        nc.tensor.matmul(
                            ps[:],
                            stat[:, k2 * 256:(k2 + 1) * 256]
                            .rearrange("p (two m) -> p two m", two=2),
                            mov,
                            start=(k2 == 0), stop=(k2 == NKT // 2 - 1),
                            perf_mode=DR)
                s_sb = gpool.tile([128, T], f32, name="s_sb", tag="s_sb",
                                  bufs=2)
                nc.scalar.activation(s_sb[:], sp[:], AF.Identity,
                                     bias=bias[:, 4:5], scale=1.0 / WSCALE)
                zbs = []
                for q in range(2):
                    zb = zpool.tile([128, T], bf16, name=f"zb{q}", tag="zb")
                    nc.scalar.activation(zb[:], zp[q][:], AF.Identity,
                                         bias=bias[:, q:q + 1],
                                         scale=1.0 / ASCALE)
                    zbs.append(zb)
                zp2 = [pp.tile([128, T], f32, space="PSUM", name=f"zp{q + 2}",
                               tag="zp", bufs=3) for q in range(2)]
                for k2 in range(NKT // 2):
                    mov = xb[:, (2 * k2) * T:(2 * k2 + 2) * T] \
                        .rearrange("p (two t) -> p two t", two=2)
                    for stat, ps in ((az[2], zp2[0]), (az[3], zp2[1])):
                        nc.tensor.matmul(
                            ps[:],
                            stat[:, k2 * 256:(k2 + 1) * 256]
                            .rearrange("p (two m) -> p two m", two=2),
                            mov,
                            start=(k2 == 0), stop=(k2 == NKT // 2 - 1),
                            perf_mode=DR)

                # ---- score transposes (PE) + stn copies (scalar) ----
                stns = []
                for ts in range(NTS):
                    s_ps = pp.tile([128, 128], f32, space="PSUM", name="s_ps",
                                   tag="tr", bufs=1)
                    nc.tensor.transpose(s_ps[:],
                                        s_sb[:, ts * 128:(ts + 1) * 128],
                                        ident[:])
                    stn = gpool.tile([128, 128], f32, name="stn", tag="stn",
                                     bufs=NTS + 1)
                    nc.scalar.copy(stn[:], s_ps[:])
                    stns.append(stn)
                state[i] = [xg, zbs, zp2, stns]

            def emit_pass1b(i):
                xg, zbs, zp2, stns = state[i]
                # ---- gating chain (DVE) ----
                # scores are shifted > 0 by the host bias, so masking out the
                # current max is just (s < r) * s (top-k/softmax are
                # shift-invariant).
                ggs = []
                for ts in range(NTS):
                    stn = stns[ts]
                    r1 = spool.tile([128, 1], f32, name="r1", tag="r1")
                    nc.vector.reduce_max(r1[:], stn[:], axis=AX.X)
                    s2 = gpool.tile([128, 128], f32, name="s2", tag="s2")
                    nc.vector.scalar_tensor_tensor(s2[:], stn[:], r1[:],
                                                   stn[:], AL.is_lt, AL.mult)
                    r2 = spool.tile([128, 1], f32, name="r2", tag="r2")
                    nc.vector.reduce_max(r2[:], s2[:], axis=AX.X)
                    s3 = gpool.tile([128, 128], f32, name="s3", tag="s3")
                    nc.vector.scalar_tensor_tensor(s3[:], s2[:], r2[:],
                                                   s2[:], AL.is_lt, AL.mult)
                    r3 = spool.tile([128, 1], f32, name="r3", tag="r3")
                    nc.vector.reduce_max(r3[:], s3[:], axis=AX.X)
                    nr1 = spool.tile([128, 1], f32, name="nr1", tag="nr1")
                    nc.vector.tensor_scalar_mul(nr1[:], r1[:], -1.0)
                    ex = gpool.tile([128, 128], f32, name="ex", tag="ex")
                    nc.scalar.activation(ex[:], stn[:], AF.Exp, bias=nr1[:],
                                         scale=1.0)
                    em = gpool.tile([128, 128], f32, name="em", tag="em")
                    zs = spool.tile([128, 1], f32, name="zs", tag="zs")
                    nc.vector.scalar_tensor_tensor(em[:], stn[:], r3[:], ex[:],
                                                   AL.is_ge, AL.mult,
                                                   accum_out=zs[:])
                    rz = spool.tile([128, 1], f32, name="rz", tag="rz")
                    nc.vector.reciprocal(rz[:], zs[:])
                    gg = gpool.tile([128, 128], f32, name="gg", tag="gg",
                                    bufs=NTS + 1)
                    nc.vector.tensor_scalar_mul(gg[:], em[:], rz[:])
                    ggs.append(gg)

                # ---- late z evictions (frees zp bufs for next mt) ----
                for q in range(2):
                    zb = zpool.tile([128, T], bf16, name=f"zb{q + 2}",
                                    tag="zb")
                    nc.scalar.activation(zb[:], zp2[q][:], AF.Identity,
                                         bias=bias[:, q + 2:q + 3],
                                         scale=1.0 / ASCALE)
                    zbs.append(zb)
                state[i] = (xg, zbs, ggs)

            def emit_pass2(i):
                xg, zbs, ggs = state.pop(i)
                npe = 0
                # ---- gate transposes back to [n, t] (bf16) ----
                gt_sb = gpool.tile([128, T], bf16, name="gt_sb", tag="gt_sb",
                                   bufs=2)
                for ts in range(NTS):
                    g_ps = pp.tile([128, 128], f32, space="PSUM", name="g_ps",
                                   tag="tr", bufs=1)
                    nc.tensor.transpose(g_ps[:], ggs[ts][:], ident[:])
                    nc.scalar.copy(gt_sb[:, ts * 128:(ts + 1) * 128], g_ps[:])

                # expands + gate-applies first so the delta stream below never
                # waits on a DVE mul queued behind residual adds
                for q in range(4):
                    gx = pp.tile([128, T], f32, space="PSUM", name="gx",
                                 tag="gx", bufs=1)
                    nc.tensor.matmul(gx[:], esb[:, q * 128:(q + 1) * 128],
                                     gt_sb[:], start=True, stop=True)
                    nc.vector.tensor_mul(zbs[q][:], zbs[q][:], gx[:])
                for q in range(4):
                    for hl in range(8):
                        hc = q * 8 + hl
                        if hl % 4 == 0:
                            yg = ypool.tile([128, 4 * T], bf16, name="yg",
                                            tag="yg")
                        ysl = yg[:, (hl % 4) * T:(hl % 4 + 1) * T]
                        xsl = xg[:, hc * T:(hc + 1) * T]
                        dp = pp.tile([128, T], f32, space="PSUM", name="dp",
                                     tag="dp", bufs=3)
                        # scalar takes the first chunks of each group while
                        # DVE is still busy with the gate multiplies
                        if hl in (0, 1, 4, 6):
                            npe += 1
                            nc.tensor.matmul(dp[:], identb[:], xsl,
                                             start=True, stop=False)
                            nc.tensor.matmul(dp[:],
                                             bpk[:, hc * 128:(hc + 1) * 128],
                                             zbs[q][:], start=False, stop=True)
                            nc.scalar.copy(ysl, dp[:])
                        else:
                            nc.tensor.matmul(dp[:],
                                             bpk[:, hc * 128:(hc + 1) * 128],
                                             zbs[q][:], start=True, stop=True)
                            nc.vector.tensor_add(ysl, xsl, dp[:])
                        if hl % 4 == 3:
                            g = hc // 4
                            row0 = (i * 8 + g) * 128
                            nst = 1
                            if i == NMT - 1:
                                nst = 4 if g == 7 else 2
                            w = 4 * T // nst
                            for hh in range(nst):
                                nc.gpsimd.dma_start(
                                    ytb_ap[row0:row0 + 128,
                                           hh * w:(hh + 1) * w],
                                    yg[:, hh * w:(hh + 1) * w])

            emit_pass1a(0)
            emit_pass1b(0)
            for i in range(1, NMT):
                emit_pass1a(i)
                emit_pass2(i - 1)
                emit_pass1b(i)
            emit_pass2(NMT - 1)

    nc.compile()
    return nc


def _prep_consts(task_emb, task_ids, Wp, bp, centers, A, Bm, adapter_scale):
    scale = float(np.asarray(adapter_scale))
    A_all = np.ascontiguousarray(
        A.transpose(1, 0, 2).reshape(H, NB * R).astype(np.float32))
    W2 = (Wp @ centers.T).astype(np.float32)                     # [H, 128]

    # ah: [p, q, hc, m] = A_all[hc*128+p, q*128+m]*ASCALE, fp8 e4m3
    ah = ((A_all * ASCALE).reshape(NKT, 128, 4, 128).transpose(1, 2, 0, 3)
          .reshape(128, 4 * NKT * 128).astype(ml_dtypes.float8_e4m3))
    ah = np.ascontiguousarray(ah)
    # w8: [p, hc, m] = W2[hc*128+p, m]*WSCALE, fp8 e4m3
    w8 = np.ascontiguousarray(
        (W2 * WSCALE).reshape(NKT, 128, 128).transpose(1, 0, 2)
        .reshape(128, NKT * 128).astype(ml_dtypes.float8_e4m3))

    # block-diag up-projection, K=128 per h-chunk
    bpk = np.zeros((128, NKT * 128), np.float32)
    for hc in range(NKT):
        for mblk in range(4):
            n = hc * 4 + mblk
            for r in range(R):
                row = (hc % 8) * 16 + mblk * 4 + r
                bpk[row, hc * 128 + mblk * 32: hc * 128 + mblk * 32 + 32] = \
                    Bm[n, r, :] * scale
    bpk = bpk.astype(ml_dtypes.bfloat16)

    e_np = (np.arange(128)[:, None] == (np.arange(512)[None, :] // 4)) \
        .astype(ml_dtypes.bfloat16)
    idb = np.eye(128, dtype=ml_dtypes.bfloat16)

    sconst = (bp @ centers.T - 0.5 * (centers ** 2).sum(-1)).astype(np.float32)

    biases = []
    for c in range(NCORES):
        te = task_emb[int(np.asarray(task_ids)[c // 2])].astype(np.float32)
        b5 = np.empty((128, 5), np.float32)
        zoff = te @ A_all                                        # [512]
        for q in range(4):
            b5[:, q] = zoff[q * 128:(q + 1) * 128]
        # +64 shifts scores strictly positive (top-k/softmax invariant);
        # the device gating chain relies on it for cheap max-masking
        b5[:, 4] = te @ W2 + sconst + 64.0
        biases.append(np.ascontiguousarray(b5))
    return ah, w8, bpk, e_np, idb, biases


def kernel(x, task_ids, task_emb, Wp, bp, centers, A, Bm, adapter_scale):
    global _COMPILED, LAST_RESULT
    from concourse import bass_utils

    x = np.asarray(x, dtype=np.float32)
    task_ids = np.asarray(task_ids)
    task_emb = np.asarray(task_emb, dtype=np.float32)
    Wp = np.asarray(Wp, dtype=np.float32)
    bp = np.asarray(bp, dtype=np.float32)
    centers = np.asarray(centers, dtype=np.float32)
    A = np.asarray(A, dtype=np.float32)
    Bm = np.asarray(Bm, dtype=np.float32)

    if _COMPILED is None:
        _COMPILED = _build()
    nc = _COMPILED

    ah, w8, bpk, e_np, idb, biases = _prep_consts(
        task_emb, task_ids, Wp, bp, centers, A, Bm, adapter_scale)

    xf = x.reshape(B * S, H)
    in_maps = []
    for c in range(NCORES):
        xtc = xf[c * TPC:(c + 1) * TPC].T                        # [H, TPC]
        # [mt][g][p][hl][t] with h = g*512 + hl*128 + p
        xtb = np.ascontiguousarray(
            xtc.reshape(8, 4, 128, NMT, T).transpose(3, 0, 2, 1, 4)
            .astype(ml_dtypes.bfloat16).reshape(NMT * 8 * 128, 4 * T))
        # [mt][g2][p][hl8][t] with h = g2*1024 + hl8*128 + p
        xt8 = np.ascontiguousarray(
            xtc.reshape(4, 8, 128, NMT, T).transpose(3, 0, 2, 1, 4)
            .astype(ml_dtypes.float8_e4m3).reshape(NMT * 4 * 128, 8 * T))
        in_maps.append({"xtb": xtb, "xt8": xt8, "ah": ah, "w8": w8,
                        "bpk": bpk, "e": e_np, "bias": biases[c], "idb": idb})

    kwargs = {}
    if TRACE:
        kwargs = dict(trace=True, tmpdir=TRACE_DIR)
    res = bass_utils.run_bass_kernel_spmd(
        nc, in_maps, core_ids=list(range(NCORES)), **kwargs)
    LAST_RESULT = res

    out = np.empty((B * S, H), np.float32)
    for c in range(NCORES):
        ytb = res.results[c]["ytb"]
        yt = (ytb.astype(np.float32).reshape(NMT, 8, 128, 4, T)
              .transpose(1, 3, 2, 0, 4).reshape(H, TPC))
        out[c * TPC:(c + 1) * TPC] = yt.T
    return out.reshape(B, S, H)
